# revision 66
# baseline (speedup 1.0000x reference)
"""DirectionalGINConv (eps=0) Trainium2 kernel, 8-core SPMD.

  agg_i = sum_{j->i} x_j ; out = relu(relu((x + agg) @ W.T + b))

Strategy (hardcoded for N=50000, E=800000, C=64, 8 cores):
  - Destination-node sharding: core c owns dst rows [c*6250, (c+1)*6250).
  - Host routes edges into per-(dst-block-of-128) tile groups of 128 edges.
    Each block b gets T_b gather tiles (uniform across cores for SPMD):
    the first K0_b tiles hold edges with src < 32768 (gather table base row
    0), the rest hold src >= 17232 (base row 17232), so gather indices fit
    in int16 (dma_gather limit). K0_b*128 is chosen so every core can route
    exactly K0_b*128 low-src edges to block b (zero pad in half0); only the
    half1 tail tile of each block carries pad slots.
  - Device per core: dma_gather x rows (fp16, rows padded to 128ch = 256B)
    block-grouped; per (chunk-of-blocks, half) the gather splits into two
    ~2.5-3k-index sub-calls round-robinned over all 4 SWDGE queues, idx
    tables DMA'd per chunk to pace dispatch. The SWDGE descriptor
    generation (Q7 CPU pair per queue, ~0.5 desc/ns aggregate across
    queues) is the hard bottleneck, not the DMA engines or HBM.
  - Segment-sum via PE: per block build one-hot S[e, j, slot] on DVE in a
    single is_equal op, then T_b accumulating matmuls
    psum[ch, slot] += G_tile.T @ S[:, j, :]; h = psum + x_shard.T (bf16);
    MLP = W.T-stationary bf16 matmul; relu+bias on ACT; PSUM->SBUF copy on
    ACT (Copy); PE transpose back to node-major; DMA out.
"""

import numpy as np
from contextlib import ExitStack

import ml_dtypes

N_NODES = 50000
IN_CH = 64
OUT_CH = 64
N_CORES = 8
SHARD = N_NODES // N_CORES          # 6250
P = 128
NBLK = (SHARD + P - 1) // P         # 49 blocks (last has 106 slots)
BASE1 = 17232                       # half1 table base (50000 - 32768)
CHUNKS = [5] * 9 + [2, 2]           # blocks per gather chunk (sum=49); each
                                    # (chunk, half) gather splits into 2
                                    # sub-calls (~2.5-3k indices each: the
                                    # SWDGE desc-gen slows beyond ~3.1k/call)


def _route(src, dst):
    """Vectorized edge routing with uniform-across-cores variable tiling.

    Returns dict with:
      K0, K1: [NBLK] int arrays (tiles per half, uniform across cores)
      idx0, idx1: [N_CORES, L0], [N_CORES, L1] int16 gather indices
      slots: [N_CORES, LT] float32 slot-in-block (-1 pad), block-tile-major
    where L0 = sum(K0)*128, L1 = sum(K1)*128, LT = L0+L1.
    """
    src = np.asarray(src, np.int64)
    dst = np.asarray(dst, np.int64)
    core = dst // SHARD
    dloc = dst - core * SHARD
    blk = dloc // P
    slot = dloc - blk * P
    gid = core * NBLK + blk
    ngrp = N_CORES * NBLK
    # categories: 0 = lo-only (half0), 1 = flexible, 2 = hi-only (half1)
    cat = np.where(src < BASE1, 0, np.where(src < 32768, 1, 2)).astype(np.int64)

    cnt = np.bincount(gid, minlength=ngrp).reshape(N_CORES, NBLK)
    n_lo = np.bincount(gid[cat == 0], minlength=ngrp).reshape(N_CORES, NBLK)
    n_flex = np.bincount(gid[cat == 1], minlength=ngrp).reshape(N_CORES, NBLK)

    # per-block uniform K0: multiple of 128 reachable by every core
    lo = n_lo.max(axis=0)                       # [NBLK] min c0 feasible all cores
    hi = (n_lo + n_flex).min(axis=0)            # [NBLK] max c0 feasible all cores
    K0 = np.zeros(NBLK, np.int64)
    K1 = np.zeros(NBLK, np.int64)
    c0 = np.zeros((N_CORES, NBLK), np.int64)
    for b in range(NBLK):
        ks = np.arange((lo[b] + 127) // 128, hi[b] // 128 + 1)
        if len(ks) > 0:
            # feasible exact multiples: choose k minimizing total tiles,
            # tie-break toward balanced halves
            tot = ks + np.maximum(0, -(-(cnt[:, b].max() - ks * 128) // 128))
            best = ks[np.lexsort((np.abs(ks * 128 - cnt[:, b].max() // 2), tot))][0]
            K0[b] = best
            c0[:, b] = best * 128
        else:  # fallback: pad in half0 too (rare/never for these sizes)
            K0[b] = -(-lo[b] // 128)
            c0[:, b] = np.minimum(K0[b] * 128, n_lo[:, b] + n_flex[:, b])
        K1[b] = max(1, int(np.max(-(-(cnt[:, b] - c0[:, b]) // 128))))
    f0 = c0 - n_lo  # flex edges sent to half0, per (core, blk)

    # rank within (gid, cat), ordered by src for gather locality
    key_gc = gid * 3 + cat
    order1 = np.lexsort((src, key_gc))
    sk = key_gc[order1]
    starts = np.r_[0, np.flatnonzero(sk[1:] != sk[:-1]) + 1]
    start_of = np.zeros(ngrp * 3, np.int64)
    start_of[sk[starts]] = starts
    rank_gc = np.empty_like(order1)
    rank_gc[order1] = np.arange(len(order1)) - start_of[key_gc][order1]

    half = np.where(cat == 0, 0,
                    np.where(cat == 2, 1,
                             (rank_gc >= f0[core, blk]).astype(np.int64)))

    # rank within (gid, half), ordered by src
    key_gh = gid * 2 + half
    order2 = np.lexsort((src, key_gh))
    sk2 = key_gh[order2]
    starts2 = np.r_[0, np.flatnonzero(sk2[1:] != sk2[:-1]) + 1]
    start_of2 = np.zeros(ngrp * 2, np.int64)
    start_of2[sk2[starts2]] = starts2
    rank = np.empty_like(order2)
    rank[order2] = np.arange(len(order2)) - start_of2[key_gh][order2]

    # layouts (uniform): per-half tile prefixes (block-major) and
    # block-tile prefixes for slots/S
    pref0 = np.r_[0, np.cumsum(K0)]             # [NBLK+1] in tiles
    pref1 = np.r_[0, np.cumsum(K1)]
    prefT = np.r_[0, np.cumsum(K0 + K1)]
    L0 = int(pref0[-1]) * P
    L1 = int(pref1[-1]) * P
    LT = int(prefT[-1]) * P

    # Spread pad indices across the table: same-address gathers serialize
    # in the SDMA path, so don't point all pads at row 0.
    spread = ((np.arange(max(L0, L1), dtype=np.int64) * 9973) % 32768).astype(np.int16)
    idx0 = np.tile(spread[:L0], (N_CORES, 1))
    idx1 = np.tile(spread[:L1], (N_CORES, 1))
    slots = np.full((N_CORES, LT), -1.0, np.float32)

    h0 = half == 0
    h1 = ~h0
    pos0 = pref0[blk[h0]] * P + rank[h0]
    pos1 = pref1[blk[h1]] * P + rank[h1]
    idx0[core[h0], pos0] = src[h0].astype(np.int16)
    idx1[core[h1], pos1] = (src[h1] - BASE1).astype(np.int16)
    # slot positions: block-tile-major, half0 tiles then half1 tiles
    spos0 = (prefT[blk[h0]] + rank[h0] // P) * P + rank[h0] % P
    spos1 = (prefT[blk[h1]] + K0[blk[h1]] + rank[h1] // P) * P + rank[h1] % P
    slots[core[h0], spos0] = slot[h0].astype(np.float32)
    slots[core[h1], spos1] = slot[h1].astype(np.float32)

    return dict(K0=K0, K1=K1, idx0=idx0, idx1=idx1, slots=slots,
                pref0=pref0, pref1=pref1, prefT=prefT)


def _wrap_idx(idx):
    """[L] int16 -> [128, L/16] wrapped (i -> [i%16, i//16]) + replicated."""
    w = idx.reshape(-1, 16).T
    return np.ascontiguousarray(np.tile(w, (8, 1)))


def _build_program(K0, K1):
    import concourse.bacc as bacc
    import concourse.tile as tile
    import concourse.mybir as mybir
    from concourse import library_config

    f16 = mybir.dt.float16
    bf16 = mybir.dt.bfloat16
    f32 = mybir.dt.float32
    i16 = mybir.dt.int16

    K0 = list(map(int, K0))
    K1 = list(map(int, K1))
    T = [a + b for a, b in zip(K0, K1)]
    TBMAX = max(T)
    pref0 = np.r_[0, np.cumsum(K0)].astype(int)
    pref1 = np.r_[0, np.cumsum(K1)].astype(int)
    prefT = np.r_[0, np.cumsum(T)].astype(int)
    L0 = int(pref0[-1]) * P
    L1 = int(pref1[-1]) * P
    assert sum(CHUNKS) == NBLK
    chunk_starts = list(np.r_[0, np.cumsum(CHUNKS)[:-1]])

    nc = bacc.Bacc("TRN2", target_bir_lowering=False, debug=False,
                   num_devices=N_CORES, num_swdge_queues=4)
    xg_d = nc.dram_tensor("xg", [N_NODES, 128], f16, kind="ExternalInput")
    i0_d = nc.dram_tensor("i0", [128, L0 // 16], i16, kind="ExternalInput")
    i1_d = nc.dram_tensor("i1", [128, L1 // 16], i16, kind="ExternalInput")
    s_d = nc.dram_tensor("s", [P, prefT[-1]], f16, kind="ExternalInput")
    xt_d = nc.dram_tensor("xt", [IN_CH, NBLK * P], f32, kind="ExternalInput")
    wt_d = nc.dram_tensor("wt", [IN_CH, OUT_CH], bf16, kind="ExternalInput")
    b_d = nc.dram_tensor("b", [OUT_CH, 1], f32, kind="ExternalInput")
    iota_d = nc.dram_tensor("iota", [P, P], f16, kind="ExternalInput")
    ident_d = nc.dram_tensor("ident", [OUT_CH, OUT_CH], f32, kind="ExternalInput")
    out_d = nc.dram_tensor("out", [SHARD, OUT_CH], f32, kind="ExternalOutput")

    with tile.TileContext(nc) as tc, ExitStack() as ctx:
        const_p = ctx.enter_context(tc.tile_pool(name="const", bufs=1))
        gat_p = ctx.enter_context(tc.tile_pool(name="gat", bufs=3))
        sel_p = ctx.enter_context(tc.tile_pool(name="sel", bufs=6))
        h_p = ctx.enter_context(tc.tile_pool(name="h", bufs=3))
        o_p = ctx.enter_context(tc.tile_pool(name="o", bufs=3))
        psum_agg = ctx.enter_context(tc.tile_pool(name="pagg", bufs=3, space="PSUM"))
        psum_mlp = ctx.enter_context(tc.tile_pool(name="pmlp", bufs=2, space="PSUM"))
        psum_tr = ctx.enter_context(tc.tile_pool(name="ptr", bufs=2, space="PSUM"))

        nc.gpsimd.load_library(library_config.mlp)

        i0_t = const_p.tile([128, L0 // 16], i16)
        i1_t = const_p.tile([128, L1 // 16], i16)
        s_t = const_p.tile([P, int(prefT[-1])], f16)
        xt_t = const_p.tile([IN_CH, NBLK * P], f32)
        wt_t = const_p.tile([IN_CH, OUT_CH], bf16)
        b_t = const_p.tile([OUT_CH, 1], f32)
        iota_t = const_p.tile([P, P], f16)
        ident_t = const_p.tile([OUT_CH, OUT_CH], f32)
        # idx tables load per-chunk on the sync queue (paces the gather
        # dispatch); other consts go via the scalar queue, the big xt last
        # so it can't gate anything early
        for t, d in [(s_t, s_d), (iota_t, iota_d), (wt_t, wt_d),
                     (b_t, b_d), (ident_t, ident_d), (xt_t, xt_d)]:
            nc.scalar.dma_start(out=t[:], in_=d.ap()[:])

        tables = [xg_d.ap()[:, :], xg_d.ap()[BASE1:, :]]
        idx_tiles = [i0_t, i1_t]
        idx_dram = [i0_d, i1_d]
        prefs = [pref0, pref1]

        qn = 0
        for ci, c0b in enumerate(chunk_starts):
            cb = CHUNKS[ci]
            t0 = [int(prefs[h][c0b]) for h in (0, 1)]
            tn = [int(prefs[h][c0b + cb]) - t0[h] for h in (0, 1)]
            for h in (0, 1):
                cA, cB_ = t0[h] * 8, (t0[h] + tn[h]) * 8
                nc.sync.dma_start(out=idx_tiles[h][:, cA:cB_],
                                  in_=idx_dram[h].ap()[:, cA:cB_])
            g = []
            for h in (0, 1):
                gt = gat_p.tile([P, tn[h], 128], f16, tag=f"g{h}",
                                name=f"g{h}_{c0b}")
                p0 = tn[h] // 2
                for off, cnt in ((0, p0), (p0, tn[h] - p0)):
                    if cnt <= 0:
                        continue
                    n_part = cnt * P
                    col0 = (t0[h] + off) * 8
                    idx_slice = idx_tiles[h][:, col0: col0 + n_part // 16]
                    nc.gpsimd.dma_gather(gt[:, off:off + cnt, :], tables[h],
                                         idx_slice, n_part, n_part, 128,
                                         single_packet=False,
                                         queue_num=qn % 4)
                    qn += 1
                g.append(gt)
            for bl in range(cb):
                blk = c0b + bl
                Tb = T[blk]
                # one-hot S for the whole block: [e, tile, slot]
                S = sel_p.tile([P, TBMAX, P], f16, name=f"S{blk}", tag="S")
                sc = int(prefT[blk])
                nc.vector.tensor_tensor(
                    out=S[:, 0:Tb, :],
                    in0=s_t[:, sc:sc + Tb][:, :, None].to_broadcast([P, Tb, P]),
                    in1=iota_t[:][:, None, :].to_broadcast([P, Tb, P]),
                    op=mybir.AluOpType.is_equal,
                )
                pa = psum_agg.tile([IN_CH, P], f32, space="PSUM")
                for j in range(Tb):
                    if j < K0[blk]:
                        gh, gidx = 0, (int(pref0[blk]) - t0[0]) + j
                    else:
                        gh, gidx = 1, (int(pref1[blk]) - t0[1]) + (j - K0[blk])
                    nc.tensor.matmul(
                        out=pa[:],
                        lhsT=g[gh][:, gidx, :IN_CH],
                        rhs=S[:, j, :],
                        start=(j == 0),
                        stop=(j == Tb - 1),
                    )
                h_t = h_p.tile([IN_CH, P], bf16)
                nc.vector.tensor_add(out=h_t[:], in0=pa[:],
                                     in1=xt_t[:, blk * P:(blk + 1) * P])
                pm = psum_mlp.tile([OUT_CH, P], f32, space="PSUM")
                nc.tensor.matmul(out=pm[:], lhsT=wt_t[:], rhs=h_t[:],
                                 start=True, stop=True)
                r_t = h_p.tile([OUT_CH, P], f32, tag="r")
                nc.scalar.activation(out=r_t[:], in_=pm[:],
                                     func=mybir.ActivationFunctionType.Relu,
                                     bias=b_t[:])
                pt = psum_tr.tile([P, OUT_CH], f32, space="PSUM")
                nc.tensor.transpose(out=pt[:], in_=r_t[:], identity=ident_t[:])
                rows = min(P, SHARD - blk * P)
                o_t = o_p.tile([P, OUT_CH], f32)
                nc.scalar.activation(out=o_t[:], in_=pt[:],
                                     func=mybir.ActivationFunctionType.Copy)
                nc.sync.dma_start(out=out_d.ap()[blk * P: blk * P + rows, :],
                                  in_=o_t[:rows, :])

    nc.compile()
    return nc


def _prepare(x, edge_index, W, b):
    """Host-side routing + per-core input maps. Returns (in_maps, route)."""
    f16np = np.float16
    x = np.asarray(x, np.float32)
    W = np.asarray(W, np.float32)
    b = np.asarray(b, np.float32)
    src = np.asarray(edge_index[0])
    dst = np.asarray(edge_index[1])

    r = _route(src, dst)
    TBMAX = int((r["K0"] + r["K1"]).max())

    xg = np.zeros((N_NODES, 128), f16np)
    xg[:, :IN_CH] = x.astype(f16np)
    iota = np.tile(np.arange(P, dtype=np.float32), (P, 1)).astype(f16np)
    ident = np.eye(OUT_CH, dtype=np.float32)
    wt = np.ascontiguousarray(W.T).astype(ml_dtypes.bfloat16)
    b2 = np.ascontiguousarray(b.reshape(-1, 1))

    in_maps = []
    for c in range(N_CORES):
        xt = np.zeros((IN_CH, NBLK * P), np.float32)
        xt[:, :SHARD] = x[c * SHARD:(c + 1) * SHARD].T
        slots = r["slots"][c]
        in_maps.append({
            "xg": xg,
            "i0": _wrap_idx(r["idx0"][c]),
            "i1": _wrap_idx(r["idx1"][c]),
            "s": np.ascontiguousarray(slots.reshape(-1, P).T).astype(f16np),
            "xt": np.ascontiguousarray(xt),
            "wt": wt,
            "b": b2,
            "iota": iota,
            "ident": ident,
        })
    return in_maps, r


_CACHE = {}


def _get_program(K0, K1):
    key = (tuple(K0), tuple(K1))
    if key not in _CACHE:
        _CACHE[key] = _build_program(K0, K1)
    return _CACHE[key]


def _best_effort_device_reset():
    """If a previous process wedged the NeuronCores, a reset lets this
    process's run succeed. Harmless (rc=0, state-free) on a healthy device."""
    try:
        import ctypes, jax
        jax.devices()
        lib = ctypes.CDLL("/opt/axon/libaxon_pjrt.so")
        lib.axon_reset.restype = ctypes.c_int64
        lib.axon_reset()
    except Exception:
        pass


def run(x, edge_index, W, b, trace=False):
    from concourse.bass_utils import run_bass_kernel_spmd
    _best_effort_device_reset()
    in_maps, r = _prepare(x, edge_index, W, b)
    nc = _get_program(r["K0"], r["K1"])
    res = run_bass_kernel_spmd(nc, in_maps, core_ids=list(range(N_CORES)),
                               trace=trace)
    out = np.concatenate([res.results[c]["out"] for c in range(N_CORES)], axis=0)
    return out.astype(np.float32), res


def kernel(x, edge_index, W, b):
    out, _ = run(x, edge_index, W, b, trace=False)
    return out


# revision 75
# speedup vs baseline: 1.0118x; 1.0118x over previous
"""DirectionalGINConv (eps=0) Trainium2 kernel, 8-core SPMD.

  agg_i = sum_{j->i} x_j ; out = relu(relu((x + agg) @ W.T + b))

Strategy (hardcoded for N=50000, E=800000, C=64, 8 cores):
  - Destination-node sharding: core c owns dst rows [c*6250, (c+1)*6250).
  - Host routes edges into per-(dst-block-of-128) tile groups of 128 edges.
    Each block b gets T_b gather tiles (uniform across cores for SPMD):
    the first K0_b tiles hold edges with src < 32768 (gather table base row
    0), the rest hold src >= 17232 (base row 17232), so gather indices fit
    in int16 (dma_gather limit). K0_b*128 is chosen so every core can route
    exactly K0_b*128 low-src edges to block b (zero pad in half0); only the
    half1 tail tile of each block carries pad slots.
  - Device per core: dma_gather x rows (fp16, rows padded to 128ch = 256B)
    block-grouped; per (chunk-of-blocks, half) the gather splits into two
    ~2.5-3k-index sub-calls round-robinned over all 4 SWDGE queues, idx
    tables DMA'd per chunk to pace dispatch. The SWDGE descriptor
    generation (Q7 CPU pair per queue, ~0.5 desc/ns aggregate across
    queues) is the hard bottleneck, not the DMA engines or HBM.
  - Segment-sum via PE: per block build one-hot S[e, j, slot] on DVE in a
    single is_equal op, then T_b accumulating matmuls
    psum[ch, slot] += G_tile.T @ S[:, j, :]; h = psum + x_shard.T (bf16);
    MLP = W.T-stationary bf16 matmul; relu+bias on ACT; PSUM->SBUF copy on
    ACT (Copy); PE transpose back to node-major; DMA out.
"""

import numpy as np
from contextlib import ExitStack

import ml_dtypes

N_NODES = 50000
IN_CH = 64
OUT_CH = 64
N_CORES = 8
SHARD = N_NODES // N_CORES          # 6250
P = 128
NBLK = (SHARD + P - 1) // P         # 49 blocks (last has 106 slots)
BASE1 = 17232                       # half1 table base (50000 - 32768)
CHUNKS = [5] * 9 + [2, 2]           # blocks per gather chunk (sum=49); each
                                    # (chunk, half) gather splits into 2
                                    # sub-calls (~2.5-3k indices each: the
                                    # SWDGE desc-gen slows beyond ~3.1k/call)


def _route(src, dst):
    """Vectorized edge routing with uniform-across-cores variable tiling.

    Returns dict with:
      K0, K1: [NBLK] int arrays (tiles per half, uniform across cores)
      idx0, idx1: [N_CORES, L0], [N_CORES, L1] int16 gather indices
      slots: [N_CORES, LT] float32 slot-in-block (-1 pad), block-tile-major
    where L0 = sum(K0)*128, L1 = sum(K1)*128, LT = L0+L1.
    """
    src = np.asarray(src, np.int64)
    dst = np.asarray(dst, np.int64)
    core = dst // SHARD
    dloc = dst - core * SHARD
    blk = dloc // P
    slot = dloc - blk * P
    gid = core * NBLK + blk
    ngrp = N_CORES * NBLK
    # categories: 0 = lo-only (half0), 1 = flexible, 2 = hi-only (half1)
    cat = np.where(src < BASE1, 0, np.where(src < 32768, 1, 2)).astype(np.int64)

    cnt = np.bincount(gid, minlength=ngrp).reshape(N_CORES, NBLK)
    n_lo = np.bincount(gid[cat == 0], minlength=ngrp).reshape(N_CORES, NBLK)
    n_flex = np.bincount(gid[cat == 1], minlength=ngrp).reshape(N_CORES, NBLK)

    # per-block uniform K0: multiple of 128 reachable by every core
    lo = n_lo.max(axis=0)                       # [NBLK] min c0 feasible all cores
    hi = (n_lo + n_flex).min(axis=0)            # [NBLK] max c0 feasible all cores
    K0 = np.zeros(NBLK, np.int64)
    K1 = np.zeros(NBLK, np.int64)
    c0 = np.zeros((N_CORES, NBLK), np.int64)
    for b in range(NBLK):
        ks = np.arange((lo[b] + 127) // 128, hi[b] // 128 + 1)
        if len(ks) > 0:
            # feasible exact multiples: choose k minimizing total tiles,
            # tie-break toward balanced halves
            tot = ks + np.maximum(0, -(-(cnt[:, b].max() - ks * 128) // 128))
            best = ks[np.lexsort((np.abs(ks * 128 - cnt[:, b].max() // 2), tot))][0]
            K0[b] = best
            c0[:, b] = best * 128
        else:  # fallback: pad in half0 too (rare/never for these sizes)
            K0[b] = -(-lo[b] // 128)
            c0[:, b] = np.minimum(K0[b] * 128, n_lo[:, b] + n_flex[:, b])
        K1[b] = max(1, int(np.max(-(-(cnt[:, b] - c0[:, b]) // 128))))
    f0 = c0 - n_lo  # flex edges sent to half0, per (core, blk)

    # rank within (gid, cat), ordered by src for gather locality
    key_gc = gid * 3 + cat
    order1 = np.lexsort((src, key_gc))
    sk = key_gc[order1]
    starts = np.r_[0, np.flatnonzero(sk[1:] != sk[:-1]) + 1]
    start_of = np.zeros(ngrp * 3, np.int64)
    start_of[sk[starts]] = starts
    rank_gc = np.empty_like(order1)
    rank_gc[order1] = np.arange(len(order1)) - start_of[key_gc][order1]

    half = np.where(cat == 0, 0,
                    np.where(cat == 2, 1,
                             (rank_gc >= f0[core, blk]).astype(np.int64)))

    # rank within (gid, half), ordered by src
    key_gh = gid * 2 + half
    order2 = np.lexsort((src, key_gh))
    sk2 = key_gh[order2]
    starts2 = np.r_[0, np.flatnonzero(sk2[1:] != sk2[:-1]) + 1]
    start_of2 = np.zeros(ngrp * 2, np.int64)
    start_of2[sk2[starts2]] = starts2
    rank = np.empty_like(order2)
    rank[order2] = np.arange(len(order2)) - start_of2[key_gh][order2]

    h0 = half == 0
    h1 = ~h0

    # half0 layout: per-block exact K0 tiles (block-major), zero pad
    pref0 = np.r_[0, np.cumsum(K0)]             # [NBLK+1] in tiles
    L0 = int(pref0[-1]) * P

    # half1 layout: POOLED per chunk — blocks' half1 edges concatenated
    # (per-core offsets), T1[chunk] = max-core ceil tiles. Saves the
    # per-block 128-rounding + cross-core max waste (~5% of descriptors).
    # Block membership is encoded in the slot value (slot + 128*b_loc) and
    # recovered by comparing against that block's iota slice; a block's
    # matmul chain covers the union tile range over cores, S masks the rest.
    nch = len(CHUNKS)
    chunk_start = np.r_[0, np.cumsum(CHUNKS)[:-1]]
    chunk_id = np.repeat(np.arange(nch), CHUNKS)
    b_loc = np.arange(NBLK) - chunk_start[chunk_id]
    c1cnt = np.bincount(gid[h1], minlength=ngrp).reshape(N_CORES, NBLK)
    assert c1cnt.min() > 0
    cum_before = np.zeros((N_CORES, NBLK), np.int64)
    T1 = np.zeros(nch, np.int64)
    for ci in range(nch):
        s, cb = int(chunk_start[ci]), CHUNKS[ci]
        cc = np.cumsum(c1cnt[:, s:s + cb], axis=1)
        cum_before[:, s:s + cb] = cc - c1cnt[:, s:s + cb]
        T1[ci] = max(1, int(-(-cc[:, -1].max() // 128)))
    t1pref = np.r_[0, np.cumsum(T1)]            # [nch+1] in tiles
    L1 = int(t1pref[-1]) * P
    first1 = (cum_before // P).min(axis=0)                        # [NBLK]
    last1 = ((cum_before + c1cnt - 1) // P).max(axis=0)           # [NBLK]

    # Spread pad indices across the table: same-address gathers serialize
    # in the SDMA path, so don't point all pads at row 0.
    spread = ((np.arange(max(L0, L1), dtype=np.int64) * 9973) % 32768).astype(np.int16)
    idx0 = np.tile(spread[:L0], (N_CORES, 1))
    idx1 = np.tile(spread[:L1], (N_CORES, 1))
    s0 = np.full((N_CORES, L0), -1.0, np.float32)
    s1 = np.full((N_CORES, L1), -1.0, np.float32)

    pos0 = pref0[blk[h0]] * P + rank[h0]
    idx0[core[h0], pos0] = src[h0].astype(np.int16)
    s0[core[h0], pos0] = slot[h0].astype(np.float32)
    pos1 = (t1pref[chunk_id[blk[h1]]] * P
            + cum_before[core[h1], blk[h1]] + rank[h1])
    idx1[core[h1], pos1] = (src[h1] - BASE1).astype(np.int16)
    s1[core[h1], pos1] = (slot[h1] + P * b_loc[blk[h1]]).astype(np.float32)

    return dict(K0=K0, T1=T1, first1=first1, last1=last1,
                idx0=idx0, idx1=idx1, s0=s0, s1=s1,
                pref0=pref0, t1pref=t1pref)


def _wrap_idx(idx):
    """[L] int16 -> [128, L/16] wrapped (i -> [i%16, i//16]) + replicated."""
    w = idx.reshape(-1, 16).T
    return np.ascontiguousarray(np.tile(w, (8, 1)))


def _build_program(K0, T1, first1, last1):
    import concourse.bacc as bacc
    import concourse.tile as tile
    import concourse.mybir as mybir
    from concourse import library_config

    f16 = mybir.dt.float16
    bf16 = mybir.dt.bfloat16
    f32 = mybir.dt.float32
    i16 = mybir.dt.int16

    K0 = list(map(int, K0))
    T1 = list(map(int, T1))
    first1 = list(map(int, first1))
    last1 = list(map(int, last1))
    R1 = [b - a + 1 for a, b in zip(first1, last1)]
    SMAX = max(k + r for k, r in zip(K0, R1))
    CBMAX = max(CHUNKS)
    pref0 = np.r_[0, np.cumsum(K0)].astype(int)
    t1pref = np.r_[0, np.cumsum(T1)].astype(int)
    L0 = int(pref0[-1]) * P
    L1 = int(t1pref[-1]) * P
    assert sum(CHUNKS) == NBLK
    chunk_starts = list(np.r_[0, np.cumsum(CHUNKS)[:-1]])

    nc = bacc.Bacc("TRN2", target_bir_lowering=False, debug=False,
                   num_devices=N_CORES, num_swdge_queues=4)
    xg_d = nc.dram_tensor("xg", [N_NODES, 128], f16, kind="ExternalInput")
    i0_d = nc.dram_tensor("i0", [128, L0 // 16], i16, kind="ExternalInput")
    i1_d = nc.dram_tensor("i1", [128, L1 // 16], i16, kind="ExternalInput")
    s0_d = nc.dram_tensor("s0", [P, L0 // P], f16, kind="ExternalInput")
    s1_d = nc.dram_tensor("s1", [P, L1 // P], f16, kind="ExternalInput")
    xt_d = nc.dram_tensor("xt", [IN_CH, NBLK * P], f32, kind="ExternalInput")
    wt_d = nc.dram_tensor("wt", [IN_CH, OUT_CH], bf16, kind="ExternalInput")
    b_d = nc.dram_tensor("b", [OUT_CH, 1], f32, kind="ExternalInput")
    iota_d = nc.dram_tensor("iota", [P, CBMAX * P], f16, kind="ExternalInput")
    ident_d = nc.dram_tensor("ident", [OUT_CH, OUT_CH], f32, kind="ExternalInput")
    out_d = nc.dram_tensor("out", [SHARD, OUT_CH], f32, kind="ExternalOutput")

    with tile.TileContext(nc) as tc, ExitStack() as ctx:
        const_p = ctx.enter_context(tc.tile_pool(name="const", bufs=1))
        gat_p = ctx.enter_context(tc.tile_pool(name="gat", bufs=3))
        sel_p = ctx.enter_context(tc.tile_pool(name="sel", bufs=6))
        h_p = ctx.enter_context(tc.tile_pool(name="h", bufs=3))
        o_p = ctx.enter_context(tc.tile_pool(name="o", bufs=3))
        psum_agg = ctx.enter_context(tc.tile_pool(name="pagg", bufs=3, space="PSUM"))
        psum_mlp = ctx.enter_context(tc.tile_pool(name="pmlp", bufs=2, space="PSUM"))
        psum_tr = ctx.enter_context(tc.tile_pool(name="ptr", bufs=2, space="PSUM"))

        nc.gpsimd.load_library(library_config.mlp)

        i0_t = const_p.tile([128, L0 // 16], i16)
        i1_t = const_p.tile([128, L1 // 16], i16)
        s0_t = const_p.tile([P, L0 // P], f16)
        s1_t = const_p.tile([P, L1 // P], f16)
        xt_t = const_p.tile([IN_CH, NBLK * P], f32)
        wt_t = const_p.tile([IN_CH, OUT_CH], bf16)
        b_t = const_p.tile([OUT_CH, 1], f32)
        iota_t = const_p.tile([P, CBMAX * P], f16)
        ident_t = const_p.tile([OUT_CH, OUT_CH], f32)
        # idx tables load per-chunk on the sync queue (paces the gather
        # dispatch); other consts go via the scalar queue, the big xt last
        # so it can't gate anything early
        for t, d in [(s0_t, s0_d), (s1_t, s1_d), (iota_t, iota_d),
                     (wt_t, wt_d), (b_t, b_d), (ident_t, ident_d),
                     (xt_t, xt_d)]:
            nc.scalar.dma_start(out=t[:], in_=d.ap()[:])

        tables = [xg_d.ap()[:, :], xg_d.ap()[BASE1:, :]]
        idx_tiles = [i0_t, i1_t]
        idx_dram = [i0_d, i1_d]

        qn = 0
        for ci, c0b in enumerate(chunk_starts):
            cb = CHUNKS[ci]
            t0 = [int(pref0[c0b]), int(t1pref[ci])]
            tn = [int(pref0[c0b + cb]) - t0[0], T1[ci]]
            for h in (0, 1):
                cA, cB_ = t0[h] * 8, (t0[h] + tn[h]) * 8
                nc.sync.dma_start(out=idx_tiles[h][:, cA:cB_],
                                  in_=idx_dram[h].ap()[:, cA:cB_])
            g = []
            for h in (0, 1):
                gt = gat_p.tile([P, tn[h], 128], f16, tag=f"g{h}",
                                name=f"g{h}_{c0b}")
                p0 = tn[h] // 2
                for off, cnt in ((0, p0), (p0, tn[h] - p0)):
                    if cnt <= 0:
                        continue
                    n_part = cnt * P
                    col0 = (t0[h] + off) * 8
                    idx_slice = idx_tiles[h][:, col0: col0 + n_part // 16]
                    nc.gpsimd.dma_gather(gt[:, off:off + cnt, :], tables[h],
                                         idx_slice, n_part, n_part, 128,
                                         single_packet=False,
                                         queue_num=qn % 4)
                    qn += 1
                g.append(gt)
            for bl in range(cb):
                blk = c0b + bl
                k0, r1 = K0[blk], R1[blk]
                Tb = k0 + r1
                # one-hot S for the block: [e, tile, slot]; half0 tiles are
                # block-exclusive (plain slots vs iota 0..127 = b_loc-0
                # slice); half1 tiles are chunk-pooled (slots offset by
                # 128*b_loc, compared vs that block's iota slice)
                S = sel_p.tile([P, SMAX, P], f16, name=f"S{blk}", tag="S")
                sc0 = int(pref0[blk])
                sc1 = int(t1pref[ci]) + first1[blk]
                nc.vector.tensor_tensor(
                    out=S[:, 0:k0, :],
                    in0=s0_t[:, sc0:sc0 + k0][:, :, None].to_broadcast([P, k0, P]),
                    in1=iota_t[:, 0:P][:, None, :].to_broadcast([P, k0, P]),
                    op=mybir.AluOpType.is_equal,
                )
                nc.vector.tensor_tensor(
                    out=S[:, k0:k0 + r1, :],
                    in0=s1_t[:, sc1:sc1 + r1][:, :, None].to_broadcast([P, r1, P]),
                    in1=iota_t[:, bl * P:(bl + 1) * P][:, None, :]
                        .to_broadcast([P, r1, P]),
                    op=mybir.AluOpType.is_equal,
                )
                pa = psum_agg.tile([IN_CH, P], f32, space="PSUM")
                for j in range(Tb):
                    if j < k0:
                        gh, gidx = 0, (int(pref0[blk]) - t0[0]) + j
                    else:
                        gh, gidx = 1, first1[blk] + (j - k0)
                    nc.tensor.matmul(
                        out=pa[:],
                        lhsT=g[gh][:, gidx, :IN_CH],
                        rhs=S[:, j, :],
                        start=(j == 0),
                        stop=(j == Tb - 1),
                    )
                h_t = h_p.tile([IN_CH, P], bf16)
                nc.vector.tensor_add(out=h_t[:], in0=pa[:],
                                     in1=xt_t[:, blk * P:(blk + 1) * P])
                pm = psum_mlp.tile([OUT_CH, P], f32, space="PSUM")
                nc.tensor.matmul(out=pm[:], lhsT=wt_t[:], rhs=h_t[:],
                                 start=True, stop=True)
                r_t = h_p.tile([OUT_CH, P], f32, tag="r")
                nc.scalar.activation(out=r_t[:], in_=pm[:],
                                     func=mybir.ActivationFunctionType.Relu,
                                     bias=b_t[:])
                pt = psum_tr.tile([P, OUT_CH], f32, space="PSUM")
                nc.tensor.transpose(out=pt[:], in_=r_t[:], identity=ident_t[:])
                rows = min(P, SHARD - blk * P)
                o_t = o_p.tile([P, OUT_CH], f32)
                nc.scalar.activation(out=o_t[:], in_=pt[:],
                                     func=mybir.ActivationFunctionType.Copy)
                nc.sync.dma_start(out=out_d.ap()[blk * P: blk * P + rows, :],
                                  in_=o_t[:rows, :])

    nc.compile()
    return nc


def _prepare(x, edge_index, W, b):
    """Host-side routing + per-core input maps. Returns (in_maps, route)."""
    f16np = np.float16
    x = np.asarray(x, np.float32)
    W = np.asarray(W, np.float32)
    b = np.asarray(b, np.float32)
    src = np.asarray(edge_index[0])
    dst = np.asarray(edge_index[1])

    r = _route(src, dst)

    xg = np.zeros((N_NODES, 128), f16np)
    xg[:, :IN_CH] = x.astype(f16np)
    # iota[p, b_loc*128 + s] = 128*b_loc + s
    iota = np.tile(np.arange(max(CHUNKS) * P, dtype=np.float32),
                   (P, 1)).astype(f16np)
    ident = np.eye(OUT_CH, dtype=np.float32)
    wt = np.ascontiguousarray(W.T).astype(ml_dtypes.bfloat16)
    b2 = np.ascontiguousarray(b.reshape(-1, 1))

    in_maps = []
    for c in range(N_CORES):
        xt = np.zeros((IN_CH, NBLK * P), np.float32)
        xt[:, :SHARD] = x[c * SHARD:(c + 1) * SHARD].T
        in_maps.append({
            "xg": xg,
            "i0": _wrap_idx(r["idx0"][c]),
            "i1": _wrap_idx(r["idx1"][c]),
            "s0": np.ascontiguousarray(
                r["s0"][c].reshape(-1, P).T).astype(f16np),
            "s1": np.ascontiguousarray(
                r["s1"][c].reshape(-1, P).T).astype(f16np),
            "xt": np.ascontiguousarray(xt),
            "wt": wt,
            "b": b2,
            "iota": iota,
            "ident": ident,
        })
    return in_maps, r


_CACHE = {}


def _get_program(K0, T1, first1, last1):
    key = (tuple(K0), tuple(T1), tuple(first1), tuple(last1))
    if key not in _CACHE:
        _CACHE[key] = _build_program(K0, T1, first1, last1)
    return _CACHE[key]


def _best_effort_device_reset():
    """If a previous process wedged the NeuronCores, a reset lets this
    process's run succeed. Harmless (rc=0, state-free) on a healthy device."""
    try:
        import ctypes, jax
        jax.devices()
        lib = ctypes.CDLL("/opt/axon/libaxon_pjrt.so")
        lib.axon_reset.restype = ctypes.c_int64
        lib.axon_reset()
    except Exception:
        pass


def run(x, edge_index, W, b, trace=False):
    from concourse.bass_utils import run_bass_kernel_spmd
    _best_effort_device_reset()
    in_maps, r = _prepare(x, edge_index, W, b)
    nc = _get_program(r["K0"], r["T1"], r["first1"], r["last1"])
    res = run_bass_kernel_spmd(nc, in_maps, core_ids=list(range(N_CORES)),
                               trace=trace)
    out = np.concatenate([res.results[c]["out"] for c in range(N_CORES)], axis=0)
    return out.astype(np.float32), res


def kernel(x, edge_index, W, b):
    out, _ = run(x, edge_index, W, b, trace=False)
    return out
